# revision 10
# baseline (speedup 1.0000x reference)
"""Trainium2 Bass kernel for causal multi-head attention (dense transformer).

Problem shapes (hardcoded): x [2,2048,1024], 16 heads x 64 head-dim.
Sharding: data-parallel over batch (2) x tensor-parallel over heads (4/core)
on 8 NeuronCores. Each core computes the partial output (sum over its 4
heads) for one batch element; the host sums the 4 partials per batch and
adds b_O.

Per-core kernel, all fp16 on-chip (host pre-casts inputs, PSUM accumulates
fp32; fp16 streams the PE at 1 cycle/col with no narrow-matmul penalty):
  - host passes x^T and pre-transposed weights as fp16; DMA queues are laid
    out so WQ/WK and the first x^T chunks land first and WV/WO trail the
    x^T stream instead of stealing its bandwidth
  - QKV projections run chunk-major (contraction-outer) so the PE starts on
    the first x^T chunk; Q/K biases ride the PSUM evacuation for free as
    per-partition tensor_scalar adds (no bias matmuls); evacuations split
    between DVE and ScalarE
  - scores are computed as S^T[k,q] (k on partitions) with the contraction
    zero-padded from 64 to 128 rows (full-array matmuls keep the PE HAM
    clock at 2.4GHz); the causal mask is applied *in PSUM* by accumulating
    IDEN^T @ TRIM(-60) onto the diagonal block, so exp underflows to 0 in
    the fp16 output and no post-exp mask multiply is needed
  - exp is fused with the PSUM->SBUF evacuation on ScalarE (fp16 out)
  - AV uses V augmented with a ones column so the softmax denominator falls
    out of the same matmul; strips are software-pipelined so the PE never
    stalls on the exp
  - normalization per (head, 512-wide q chunk) as soon as that chunk's AV
    accumulation finishes: one [1,512] denominator-row copy,
    reciprocal_approx_fast, gpsimd partition_broadcast, and a single fused
    [64,512] tensor_tensor multiply that evacuates-and-normalizes z into
    fp16 ZN
  - output projection is interleaved into the attention stream as q-blocks
    of ZN complete, borrowing score-strip PSUM tiles (stays within 8 PSUM
    banks) so the PE never idles and the HAM clock stays at 2.4GHz; partial
    outputs leave as fp16 (host sums in fp32)
"""

import sys

if "/opt/trn_rl_repo" not in sys.path:
    sys.path.insert(0, "/opt/trn_rl_repo")

import numpy as np

B, S, D = 2, 2048, 1024
H, DH = 16, 64
NCORES = 8
NH = 4            # heads per core
KCH = D // 128    # contraction chunks over model dim
NT = S // 128     # 128-row tiles over sequence
QC = S // 512     # 512-wide q chunks
P = 128
MASK_VAL = -60.0

_CACHE = {}


def _build_nc(debug=False):
    import concourse.tile as tile
    from concourse import bacc, mybir

    f32 = mybir.dt.float32
    f16 = mybir.dt.float16
    Exp = mybir.ActivationFunctionType.Exp
    mult = mybir.AluOpType.mult

    nc = bacc.Bacc("TRN2", target_bir_lowering=False, debug=False,
                   num_devices=NCORES)

    xt_d = nc.dram_tensor("xt", [D, S], f16, kind="ExternalInput").ap()
    wq_d = nc.dram_tensor("wq", [P, KCH * NH * DH], f16, kind="ExternalInput").ap()
    wk_d = nc.dram_tensor("wk", [P, KCH * NH * DH], f16, kind="ExternalInput").ap()
    wv_d = nc.dram_tensor("wv", [P, KCH * NH * DH], f16, kind="ExternalInput").ap()
    wo_d = nc.dram_tensor("wo", [P, 2 * D], f16, kind="ExternalInput").ap()
    bqc_d = nc.dram_tensor("bqc", [P, 2], f32, kind="ExternalInput").ap()
    bkc_d = nc.dram_tensor("bkc", [P, 2], f32, kind="ExternalInput").ap()
    bv_d = nc.dram_tensor("bv", [1, NH * DH], f16, kind="ExternalInput").ap()
    ones_d = nc.dram_tensor("ones", [1, S], f16, kind="ExternalInput").ap()
    vones_d = nc.dram_tensor("vones", [P, NT * NH], f16, kind="ExternalInput").ap()
    trim_d = nc.dram_tensor("trim", [P, P], f16, kind="ExternalInput").ap()
    iden_d = nc.dram_tensor("iden", [P, P], f16, kind="ExternalInput").ap()
    out_d = nc.dram_tensor("out", [S, D], f16, kind="ExternalOutput").ap()
    dbg = {}
    if debug:
        dbg["qt"] = nc.dram_tensor("dbg_qt", [P, 2 * S], f16, kind="ExternalOutput").ap()
        dbg["kt"] = nc.dram_tensor("dbg_kt", [P, NH * S], f16, kind="ExternalOutput").ap()
        dbg["v"] = nc.dram_tensor("dbg_v", [P, NT * NH * (DH + 1)], f16, kind="ExternalOutput").ap()
        dbg["zn"] = nc.dram_tensor("dbg_zn", [P, 2 * S], f16, kind="ExternalOutput").ap()
        dbg["es"] = nc.dram_tensor("dbg_es", [P, 1024], f16, kind="ExternalOutput").ap()

    with tile.TileContext(nc) as tc:
        from contextlib import ExitStack

        with ExitStack() as ctx:
            persist = ctx.enter_context(tc.tile_pool(name="persist", bufs=1))

            XT = persist.tile([P, KCH, S], f16)
            QT = persist.tile([P, 2, S], f16)
            KT = persist.tile([P, NH, S], f16)
            V = persist.tile([P, NT, NH, DH + 1], f16)
            ZN = persist.tile([P, 2, S], f16)
            WQ = persist.tile([P, KCH, NH * DH], f16)
            WK = persist.tile([P, KCH, NH * DH], f16)
            WV = persist.tile([P, KCH, NH * DH], f16)
            WO = persist.tile([P, 2, D], f16)
            BQC = persist.tile([P, 2], f32)
            BKC = persist.tile([P, 2], f32)
            BV = persist.tile([1, NH * DH], f16)
            ONES = persist.tile([1, S], f16)
            TRIM = persist.tile([P, P], f16)
            IDEN = persist.tile([P, P], f16)
            WARM = persist.tile([P, P], f16)

            # ---- PE warmup on a memset tile (no DMA dependency) ----
            nc.vector.memset(WARM, 0.0)
            with tc.tile_pool(name="warm_ps", bufs=1, space="PSUM") as wp:
                wps = wp.tile([P, P], f32)
                for _ in range(36):
                    nc.tensor.matmul(wps, WARM, WARM, start=True, stop=True)

            # preload the Exp activation table while DMAs stream
            with tc.tile_pool(name="pre", bufs=1) as pre:
                dumb = pre.tile([1, P], f16)
                nc.scalar.activation(dumb, WARM[0:1, :], Exp)

            # zero the pad half of KT (head h occupies partitions
            # (h%2)*64 .. +64 of column-block h; the rest must be 0)
            nc.vector.memset(KT.rearrange("p a b -> p (a b)"), 0.0)

            # ---- input DMAs (already fp16 on host) ----
            # gpsimd: small tensors + QK weights (arrive ~3us)
            nc.gpsimd.dma_start(WQ.rearrange("p a b -> p (a b)"), wq_d)
            nc.gpsimd.dma_start(BQC, bqc_d)
            nc.gpsimd.dma_start(BKC, bkc_d)
            nc.gpsimd.dma_start(BV, bv_d)
            nc.gpsimd.dma_start(ONES, ones_d)
            nc.gpsimd.dma_start(WK.rearrange("p a b -> p (a b)"), wk_d)
            # sync/scalar: x^T chunks; WV/vones/WO trail them (needed later)
            nc.scalar.dma_start(TRIM, trim_d)
            nc.scalar.dma_start(IDEN, iden_d)
            for ch in range(KCH):
                eng = nc.sync if ch % 2 == 0 else nc.scalar
                eng.dma_start(XT[:, ch, :],
                              xt_d[ch * P:(ch + 1) * P, :])
            nc.sync.dma_start(WV.rearrange("p a b -> p (a b)"), wv_d)
            nc.sync.dma_start(V[:, :, :, DH:DH + 1], vones_d)
            nc.scalar.dma_start(WO.rearrange("p a b -> p (a b)"), wo_d)

            def qk_sweep(qkv_ps, sweep):
                pst = {}
                for wi in range(2):
                    for t in range(2):
                        for qc in (2 * sweep, 2 * sweep + 1):
                            pst[(wi, t, qc)] = qkv_ps.tile(
                                [P, 512], f32, tag="qk",
                                name=f"qk{sweep}_{wi}_{t}_{qc}")
                for ch in range(KCH):
                    for wi, W_ in enumerate((WQ, WK)):
                        for t in range(2):
                            for qc in (2 * sweep, 2 * sweep + 1):
                                nc.tensor.matmul(
                                    pst[(wi, t, qc)],
                                    W_[:, ch, t * P:(t + 1) * P],
                                    XT[:, ch, qc * 512:(qc + 1) * 512],
                                    start=(ch == 0), stop=(ch == KCH - 1))
                for wi, B_ in enumerate((BQC, BKC)):
                    for t in range(2):
                        for qc in (2 * sweep, 2 * sweep + 1):
                            ps = pst[(wi, t, qc)]
                            sl = slice(qc * 512, (qc + 1) * 512)
                            # bias rides the evacuation as a per-partition add
                            if wi == 0:
                                nc.vector.tensor_scalar_add(
                                    QT[:, t, sl], ps, B_[:, t:t + 1])
                            else:
                                nc.vector.tensor_scalar_add(
                                    KT[0:64, 2 * t, sl], ps[0:64, :],
                                    B_[0:64, t:t + 1])
                                nc.vector.tensor_scalar_add(
                                    KT[64:128, 2 * t + 1, sl], ps[64:128, :],
                                    B_[64:128, t:t + 1])

            # ---- phase 1: Q/K projections + all of V ----
            with tc.tile_pool(name="qkv_ps", bufs=8, space="PSUM") as qkv_ps:
                qk_sweep(qkv_ps, 0)
                qk_sweep(qkv_ps, 1)
                for vs in range(2):
                    psv = [qkv_ps.tile([P, 512], f32, tag="qk",
                                       name=f"v_{vs}_{i}")
                           for i in range(KCH)]
                    for ch in range(KCH):
                        for i in range(KCH):
                            kt = vs * KCH + i
                            nc.tensor.matmul(
                                psv[i][:, 0:NH * DH],
                                XT[:, ch, kt * P:(kt + 1) * P],
                                WV[:, ch, :], start=(ch == 0), stop=False)
                    for i in range(KCH):
                        kt = vs * KCH + i
                        nc.tensor.matmul(
                            psv[i][:, 0:NH * DH],
                            ONES[:, kt * P:(kt + 1) * P], BV,
                            start=False, stop=True)
                        # split V evacuations between ScalarE and DVE
                        if i % 2 == 0:
                            nc.scalar.copy(
                                V[:, kt, :, 0:DH], psv[i][:, 0:NH * DH])
                        else:
                            nc.vector.tensor_copy(
                                V[:, kt, :, 0:DH], psv[i][:, 0:NH * DH])

            # ---- phase 2: attention + interleaved output projection ----
            with tc.tile_pool(name="esp", bufs=4) as esp, \
                    tc.tile_pool(name="nrm", bufs=4) as nrm, \
                    tc.tile_pool(name="rrb", bufs=4) as rrb:

                def emit_scores(sc_ps, h, kb, hf):
                    t = h // 2
                    k0 = kb * P
                    hstart = hf * 1024
                    qstart = max(k0, hstart)
                    strip_ps = sc_ps.tile([P, 1024], f32,
                                          name=f"sps_{h}_{kb}_{hf}", tag="sps")
                    strip_sb = esp.tile([P, 1024], f16,
                                        name=f"ssb_{h}_{kb}_{hf}", tag="ssb")
                    has_diag = k0 >= hstart
                    qpos = qstart
                    first = True
                    while qpos < hstart + 1024:
                        qnext = min(hstart + 1024, (qpos // 512 + 1) * 512)
                        nc.tensor.matmul(
                            strip_ps[:, qpos - hstart:qnext - hstart],
                            KT[:, h, k0:k0 + P],
                            QT[:, t, qpos:qnext],
                            start=True, stop=not (has_diag and first))
                        if has_diag and first:
                            # accumulate IDEN^T @ TRIM = TRIM onto the
                            # diagonal block (always inside the first chunk)
                            # so exp underflows to 0 above the diagonal
                            dsl = slice(k0 - hstart, k0 - hstart + P)
                            nc.tensor.matmul(
                                strip_ps[:, dsl], IDEN, TRIM,
                                start=False, stop=True, skip_group_check=True)
                        first = False
                        qpos = qnext
                    nc.scalar.activation(
                        strip_sb[:, qstart - hstart:1024],
                        strip_ps[:, qstart - hstart:1024], Exp)
                    if debug and h == 0 and kb == 0 and hf == 0:
                        nc.gpsimd.dma_start(dbg["es"], strip_sb)
                    return strip_sb

                avs = {}

                def emit_av(av_ps, h, kb, hf, strip_sb):
                    t, pb = h // 2, (h % 2) * 64
                    k0 = kb * P
                    hstart = hf * 1024
                    qstart = max(k0, hstart)
                    if kb == 0:
                        avs[(h, hf)] = av_ps.tile(
                            [DH + 1, 2, 512], f32,
                            tag="av", name=f"av_{h}_{hf}")
                    av = avs[(h, hf)]
                    qpos = qstart
                    while qpos < hstart + 1024:
                        qc = qpos // 512
                        qnext = min(hstart + 1024, (qc + 1) * 512)
                        done = kb == 4 * qc + 3
                        qr = qc - 2 * hf
                        nc.tensor.matmul(
                            av[:, qr, qpos - qc * 512:qnext - qc * 512],
                            V[:, kb, h, :],
                            strip_sb[:, qpos - hstart:qnext - hstart],
                            start=(kb == 0), stop=done)
                        if done:
                            # normalize this 512-wide q chunk now
                            drow = nrm.tile([1, 512], f32, tag="dr")
                            nc.vector.tensor_copy(drow, av[DH:DH + 1, qr, :])
                            rr = nrm.tile([1, 512], f32, tag="rr")
                            nc.vector.reciprocal_approx_fast(out=rr, in_=drow)
                            rb = rrb.tile([64, 512], f32, tag="rb")
                            nc.gpsimd.partition_broadcast(rb, rr)
                            nc.vector.tensor_tensor(
                                ZN[pb:pb + 64, t, qc * 512:(qc + 1) * 512],
                                av[0:DH, qr, :], rb, mult)
                        qpos = qnext

                from collections import deque

                def emit_op(sc_ps, osb, qt):
                    # output projection for one 128-row q block, borrowing a
                    # score-strip PSUM tile (keeps total PSUM at 8 banks)
                    ps = sc_ps.tile([P, 1024], f32, tag="sps",
                                    name=f"op_{qt}")
                    for dc in range(2):
                        for t in range(2):
                            nc.tensor.matmul(
                                ps[:, dc * 512:(dc + 1) * 512],
                                ZN[:, t, qt * P:(qt + 1) * P],
                                WO[:, t, dc * 512:(dc + 1) * 512],
                                start=(t == 0), stop=(t == 1))
                    ob = osb.tile([P, 1024], f16, tag="ob",
                                  name=f"ob_{qt}")
                    nc.vector.tensor_copy(ob, ps)
                    oeng = (nc.sync, nc.scalar, nc.gpsimd)[qt % 3]
                    oeng.dma_start(out_d[qt * P:(qt + 1) * P, :], ob)

                def run_strips(sc_ps, av_ps, osb, work):
                    pending = deque()
                    for item in work:
                        if item[0] == "op":
                            emit_op(sc_ps, osb, item[1])
                            continue
                        sid = item[1]
                        sb_tile = emit_scores(sc_ps, *sid)
                        pending.append((sid, sb_tile))
                        if len(pending) > 2:
                            psid, psb = pending.popleft()
                            emit_av(av_ps, *psid, psb)
                    while pending:
                        psid, psb = pending.popleft()
                        emit_av(av_ps, *psid, psb)

                # hf-major strip order: all q-half-0 strips first so
                # ZN[:, :, 0:1024] completes early; out-proj q-blocks are
                # interleaved as soon as their ZN rows are final.
                work = []
                for h in range(NH):
                    for kb in range(QC + 4):
                        work.append(("sc", (h, kb, 0)))
                work.append(("op", 0))
                work.append(("op", 1))
                for h in range(NH):
                    for kb in range(NT):
                        work.append(("sc", (h, kb, 1)))
                        # h3: the qc2 norm is emitted when the kb11 strip is
                        # popped (at the kb13 push); qt 8..11 only need
                        # ZN q 1024:1536
                        if h == NH - 1 and kb >= 13:
                            work.append(("op", kb - 5))
                    if h < 3:
                        work.append(("op", 2 + 2 * h))
                        work.append(("op", 3 + 2 * h))
                work.append(("op", 11))

                with tc.tile_pool(name="sc_psA", bufs=2, space="PSUM") as scA, \
                        tc.tile_pool(name="av_psA", bufs=2, space="PSUM") as avA, \
                        tc.tile_pool(name="osb", bufs=3) as osb:
                    run_strips(scA, avA, osb, work)
                    if debug:
                        for nm, tl in (("qt", QT), ("kt", KT),
                                       ("v", V), ("zn", ZN)):
                            nc.gpsimd.dma_start(
                                dbg[nm], tl.rearrange("p ... -> p (...)"))
                    # remaining out-projection (q 1536:2048)
                    for qt in range(12, NT):
                        emit_op(scA, osb, qt)

    nc.compile()
    return nc


def _get_nc(debug=False):
    key = ("nc", debug)
    if key not in _CACHE:
        _CACHE[key] = _build_nc(debug)
    return _CACHE[key]


def _host_inputs(x, W_Q, W_K, W_V, W_O, b_Q, b_K, b_V):
    """Build the 8 per-core input maps (all fp16)."""
    x = np.asarray(x, dtype=np.float32)
    scale = 1.0 / np.sqrt(np.float32(DH))
    ones = np.ones((1, S), dtype=np.float16)
    vones = np.ones((P, NT * NH), dtype=np.float16)
    trim = np.where(np.arange(P)[:, None] <= np.arange(P)[None, :],
                    np.float32(0.0), np.float32(MASK_VAL)).astype(np.float16)
    iden = np.eye(P, dtype=np.float16)

    xts = [np.ascontiguousarray(x[b].T).astype(np.float16) for b in range(B)]

    in_maps = []
    for c in range(NCORES):
        b, hg = divmod(c, NCORES // B)
        h0 = NH * hg
        def chunked(a):   # [D, M] -> [128, KCH*M] with rows p, cols (ch, m)
            return np.ascontiguousarray(
                a.reshape(KCH, P, -1).transpose(1, 0, 2).reshape(P, -1)
            ).astype(np.float16)
        wq = chunked((np.asarray(W_Q[h0:h0 + NH], np.float32) * scale)
                     .reshape(NH * DH, D).T)
        wk = chunked(np.asarray(W_K[h0:h0 + NH], np.float32)
                     .reshape(NH * DH, D).T)
        wv = chunked(np.asarray(W_V[h0:h0 + NH], np.float32)
                     .reshape(NH * DH, D).T)
        wo_flat = np.asarray(W_O[h0:h0 + NH], np.float32) \
            .transpose(0, 2, 1).reshape(NH * DH, D)
        wo = np.ascontiguousarray(
            wo_flat.reshape(2, P, D).transpose(1, 0, 2).reshape(P, 2 * D)
        ).astype(np.float16)
        # per-partition bias columns: col t = heads (2t, 2t+1) x 64 dh
        bqc = np.ascontiguousarray(
            (np.asarray(b_Q[h0:h0 + NH], np.float32) * scale)
            .reshape(2, P).T).astype(np.float32)
        bkc = np.ascontiguousarray(
            np.asarray(b_K[h0:h0 + NH], np.float32)
            .reshape(2, P).T).astype(np.float32)
        bv = np.asarray(b_V[h0:h0 + NH], np.float32) \
            .reshape(1, NH * DH).astype(np.float16)
        in_maps.append({
            "xt": xts[b], "wq": wq, "wk": wk, "wv": wv, "wo": wo,
            "bqc": bqc, "bkc": bkc,
            "bv": np.ascontiguousarray(bv), "ones": ones, "vones": vones,
            "trim": trim, "iden": iden,
        })
    return in_maps


def run_spmd(in_maps, debug=False, **kwargs):
    from concourse import bass_utils
    nc = _get_nc(debug)
    return bass_utils.run_bass_kernel_spmd(
        nc, in_maps, core_ids=list(range(NCORES)), **kwargs)


def kernel(x, W_Q, W_K, W_V, W_O, b_Q, b_K, b_V, b_O):
    in_maps = _host_inputs(x, W_Q, W_K, W_V, W_O, b_Q, b_K, b_V)
    res = run_spmd(in_maps)
    parts = [res.results[c]["out"].astype(np.float32) for c in range(NCORES)]
    gpb = NCORES // B
    out = np.stack(
        [sum(parts[b * gpb + g] for g in range(gpb)) for b in range(B)], axis=0)
    out += np.asarray(b_O, np.float32)[None, None, :]
    return out.astype(np.float32)


# revision 14
# speedup vs baseline: 1.0851x; 1.0851x over previous
"""Trainium2 Bass kernel for causal multi-head attention (dense transformer).

Problem shapes (hardcoded): x [2,2048,1024], 16 heads x 64 head-dim.
Sharding: data-parallel over batch (2) x tensor-parallel over heads (4/core)
on 8 NeuronCores. Each core computes the partial output (sum over its 4
heads) for one batch element; the host sums the 4 partials per batch and
adds b_O.

Per-core kernel, all fp16 on-chip (host pre-casts inputs, PSUM accumulates
fp32; fp16 streams the PE at 1 cycle/col with no narrow-matmul penalty):
  - host passes x^T and pre-transposed weights as fp16; DMA queues are laid
    out so WQ/WK and the first x^T chunks land first and WV/WO trail the
    x^T stream instead of stealing its bandwidth
  - QKV projections run chunk-major (contraction-outer) so the PE starts on
    the first x^T chunk; Q/K biases ride the PSUM evacuation for free as
    per-partition tensor_scalar adds (no bias matmuls); evacuations split
    between DVE and ScalarE
  - scores are computed as S^T[k,q] (k on partitions) with the contraction
    zero-padded from 64 to 128 rows (full-array matmuls keep the PE HAM
    clock at 2.4GHz); the causal mask is applied *in PSUM* by accumulating
    IDEN^T @ TRIM(-60) onto the diagonal block, so exp underflows to 0 in
    the fp16 output and no post-exp mask multiply is needed
  - exp is fused with the PSUM->SBUF evacuation on ScalarE (fp16 out)
  - AV uses V augmented with a ones column so the softmax denominator falls
    out of the same matmul; strips are software-pipelined so the PE never
    stalls on the exp
  - normalization per (head, 512-wide q chunk) as soon as that chunk's AV
    accumulation finishes: one [1,512] denominator-row copy,
    reciprocal_approx_fast, gpsimd partition_broadcast, and a single fused
    [64,512] tensor_tensor multiply that evacuates-and-normalizes z into
    fp16 ZN
  - output projection is interleaved into the attention stream as q-blocks
    of ZN complete, borrowing score-strip PSUM tiles (stays within 8 PSUM
    banks) so the PE never idles and the HAM clock stays at 2.4GHz; partial
    outputs leave as fp16 (host sums in fp32)
"""

import sys

if "/opt/trn_rl_repo" not in sys.path:
    sys.path.insert(0, "/opt/trn_rl_repo")

import numpy as np

B, S, D = 2, 2048, 1024
H, DH = 16, 64
NCORES = 8
NH = 4            # heads per core
KCH = D // 128    # contraction chunks over model dim
NT = S // 128     # 128-row tiles over sequence
QC = S // 512     # 512-wide q chunks
P = 128
MASK_VAL = -60.0

_CACHE = {}


def _build_nc(debug=False):
    import concourse.tile as tile
    from concourse import bacc, mybir

    f32 = mybir.dt.float32
    f16 = mybir.dt.float16
    Exp = mybir.ActivationFunctionType.Exp
    mult = mybir.AluOpType.mult

    nc = bacc.Bacc("TRN2", target_bir_lowering=False, debug=False,
                   num_devices=NCORES)

    xt_d = nc.dram_tensor("xt", [D, S], f16, kind="ExternalInput").ap()
    wq_d = nc.dram_tensor("wq", [P, KCH * NH * DH], f16, kind="ExternalInput").ap()
    wk_d = nc.dram_tensor("wk", [P, KCH * NH * DH], f16, kind="ExternalInput").ap()
    wv_d = nc.dram_tensor("wv", [P, KCH * NH * DH], f16, kind="ExternalInput").ap()
    wo_d = nc.dram_tensor("wo", [P, 2 * D], f16, kind="ExternalInput").ap()
    bqc_d = nc.dram_tensor("bqc", [P, 2], f32, kind="ExternalInput").ap()
    bkc_d = nc.dram_tensor("bkc", [P, 2], f32, kind="ExternalInput").ap()
    bv_d = nc.dram_tensor("bv", [1, NH * DH], f16, kind="ExternalInput").ap()
    ones_d = nc.dram_tensor("ones", [1, S], f16, kind="ExternalInput").ap()
    vones_d = nc.dram_tensor("vones", [P, NT * NH], f16, kind="ExternalInput").ap()
    trim_d = nc.dram_tensor("trim", [P, P], f16, kind="ExternalInput").ap()
    iden_d = nc.dram_tensor("iden", [P, P], f16, kind="ExternalInput").ap()
    out_d = nc.dram_tensor("out", [S, D], f16, kind="ExternalOutput").ap()
    dbg = {}
    if debug:
        dbg["qt"] = nc.dram_tensor("dbg_qt", [P, 2 * S], f16, kind="ExternalOutput").ap()
        dbg["kt"] = nc.dram_tensor("dbg_kt", [P, NH * S], f16, kind="ExternalOutput").ap()
        dbg["v"] = nc.dram_tensor("dbg_v", [P, NT * NH * (DH + 1)], f16, kind="ExternalOutput").ap()
        dbg["zn"] = nc.dram_tensor("dbg_zn", [P, 2 * S], f16, kind="ExternalOutput").ap()
        dbg["es"] = nc.dram_tensor("dbg_es", [P, 1024], f16, kind="ExternalOutput").ap()

    with tile.TileContext(nc) as tc:
        from contextlib import ExitStack

        with ExitStack() as ctx:
            persist = ctx.enter_context(tc.tile_pool(name="persist", bufs=1))

            XT = persist.tile([P, KCH, S], f16)
            QT = persist.tile([P, 2, S], f16)
            KT = persist.tile([P, NH, S], f16)
            V = persist.tile([P, NT, NH, DH + 1], f16)
            ZN = persist.tile([P, 2, S], f16)
            WQ = persist.tile([P, KCH, NH * DH], f16)
            WK = persist.tile([P, KCH, NH * DH], f16)
            WV = persist.tile([P, KCH, NH * DH], f16)
            WO = persist.tile([P, 2, D], f16)
            BQC = persist.tile([P, 2], f32)
            BKC = persist.tile([P, 2], f32)
            BV = persist.tile([1, NH * DH], f16)
            ONES = persist.tile([1, S], f16)
            TRIM = persist.tile([P, P], f16)
            IDEN = persist.tile([P, P], f16)
            WARM = persist.tile([P, P], f16)

            # ---- PE warmup on a memset tile (no DMA dependency) ----
            nc.vector.memset(WARM, 0.0)
            with tc.tile_pool(name="warm_ps", bufs=1, space="PSUM") as wp:
                wps = wp.tile([P, P], f32)
                for _ in range(28):
                    nc.tensor.matmul(wps, WARM, WARM, start=True, stop=True)

            # preload the Exp activation table while DMAs stream
            with tc.tile_pool(name="pre", bufs=1) as pre:
                dumb = pre.tile([1, P], f16)
                nc.scalar.activation(dumb, WARM[0:1, :], Exp)

            # zero the pad half of KT (head h occupies partitions
            # (h%2)*64 .. +64 of column-block h; the rest must be 0)
            nc.vector.memset(KT.rearrange("p a b -> p (a b)"), 0.0)

            # ---- input DMAs (already fp16 on host) ----
            # gpsimd: small tensors + QK weights (arrive ~3us)
            nc.gpsimd.dma_start(WQ.rearrange("p a b -> p (a b)"), wq_d)
            nc.gpsimd.dma_start(BQC, bqc_d)
            nc.gpsimd.dma_start(BKC, bkc_d)
            nc.gpsimd.dma_start(BV, bv_d)
            nc.gpsimd.dma_start(ONES, ones_d)
            nc.gpsimd.dma_start(WK.rearrange("p a b -> p (a b)"), wk_d)
            # sync/scalar: x^T chunks; WV/vones/WO trail them (needed later)
            nc.scalar.dma_start(TRIM, trim_d)
            nc.scalar.dma_start(IDEN, iden_d)
            for ch in range(KCH):
                eng = nc.sync if ch % 2 == 0 else nc.scalar
                eng.dma_start(XT[:, ch, :],
                              xt_d[ch * P:(ch + 1) * P, :])
            nc.sync.dma_start(WV.rearrange("p a b -> p (a b)"), wv_d)
            nc.sync.dma_start(V[:, :, :, DH:DH + 1], vones_d)
            nc.scalar.dma_start(WO.rearrange("p a b -> p (a b)"), wo_d)

            def qk_sweep(qkv_ps, sweep):
                pst = {}
                for wi in range(2):
                    for t in range(2):
                        for qc in (2 * sweep, 2 * sweep + 1):
                            pst[(wi, t, qc)] = qkv_ps.tile(
                                [P, 512], f32, tag="qk",
                                name=f"qk{sweep}_{wi}_{t}_{qc}")
                for ch in range(KCH):
                    for wi, W_ in enumerate((WQ, WK)):
                        for t in range(2):
                            for qc in (2 * sweep, 2 * sweep + 1):
                                nc.tensor.matmul(
                                    pst[(wi, t, qc)],
                                    W_[:, ch, t * P:(t + 1) * P],
                                    XT[:, ch, qc * 512:(qc + 1) * 512],
                                    start=(ch == 0), stop=(ch == KCH - 1))
                for wi, B_ in enumerate((BQC, BKC)):
                    for t in range(2):
                        for qc in (2 * sweep, 2 * sweep + 1):
                            ps = pst[(wi, t, qc)]
                            sl = slice(qc * 512, (qc + 1) * 512)
                            # bias rides the evacuation as a per-partition add
                            if wi == 0:
                                nc.vector.tensor_scalar_add(
                                    QT[:, t, sl], ps, B_[:, t:t + 1])
                            else:
                                nc.vector.tensor_scalar_add(
                                    KT[0:64, 2 * t, sl], ps[0:64, :],
                                    B_[0:64, t:t + 1])
                                nc.vector.tensor_scalar_add(
                                    KT[64:128, 2 * t + 1, sl], ps[64:128, :],
                                    B_[64:128, t:t + 1])

            # ---- phase 1: Q/K projections + all of V ----
            with tc.tile_pool(name="qkv_ps", bufs=8, space="PSUM") as qkv_ps:
                qk_sweep(qkv_ps, 0)
                qk_sweep(qkv_ps, 1)
                for vs in range(2):
                    psv = [qkv_ps.tile([P, 512], f32, tag="qk",
                                       name=f"v_{vs}_{i}")
                           for i in range(KCH)]
                    for ch in range(KCH):
                        for i in range(KCH):
                            kt = vs * KCH + i
                            nc.tensor.matmul(
                                psv[i][:, 0:NH * DH],
                                XT[:, ch, kt * P:(kt + 1) * P],
                                WV[:, ch, :], start=(ch == 0), stop=False)
                    for i in range(KCH):
                        kt = vs * KCH + i
                        nc.tensor.matmul(
                            psv[i][:, 0:NH * DH],
                            ONES[:, kt * P:(kt + 1) * P], BV,
                            start=False, stop=True)
                        # split V evacuations between ScalarE and DVE
                        if i % 2 == 0:
                            nc.scalar.copy(
                                V[:, kt, :, 0:DH], psv[i][:, 0:NH * DH])
                        else:
                            nc.vector.tensor_copy(
                                V[:, kt, :, 0:DH], psv[i][:, 0:NH * DH])

            # ---- phase 2: attention + interleaved output projection ----
            with tc.tile_pool(name="esp", bufs=4) as esp, \
                    tc.tile_pool(name="nrm", bufs=4) as nrm, \
                    tc.tile_pool(name="rrb", bufs=4) as rrb:

                def emit_scores(sc_ps, h, kb, hf):
                    t = h // 2
                    k0 = kb * P
                    hstart = hf * 1024
                    qstart = max(k0, hstart)
                    strip_ps = sc_ps.tile([P, 1024], f32,
                                          name=f"sps_{h}_{kb}_{hf}", tag="sps")
                    strip_sb = esp.tile([P, 1024], f16,
                                        name=f"ssb_{h}_{kb}_{hf}", tag="ssb")
                    has_diag = k0 >= hstart
                    qpos = qstart
                    first = True
                    while qpos < hstart + 1024:
                        qnext = min(hstart + 1024, (qpos // 512 + 1) * 512)
                        nc.tensor.matmul(
                            strip_ps[:, qpos - hstart:qnext - hstart],
                            KT[:, h, k0:k0 + P],
                            QT[:, t, qpos:qnext],
                            start=True, stop=not (has_diag and first))
                        if has_diag and first:
                            # accumulate IDEN^T @ TRIM = TRIM onto the
                            # diagonal block (always inside the first chunk)
                            # so exp underflows to 0 above the diagonal
                            dsl = slice(k0 - hstart, k0 - hstart + P)
                            nc.tensor.matmul(
                                strip_ps[:, dsl], IDEN, TRIM,
                                start=False, stop=True, skip_group_check=True)
                        first = False
                        qpos = qnext
                    nc.scalar.activation(
                        strip_sb[:, qstart - hstart:1024],
                        strip_ps[:, qstart - hstart:1024], Exp)
                    if debug and h == 0 and kb == 0 and hf == 0:
                        nc.gpsimd.dma_start(dbg["es"], strip_sb)
                    return strip_sb

                avs = {}

                def emit_av(av_ps, h, kb, hf, strip_sb):
                    t, pb = h // 2, (h % 2) * 64
                    k0 = kb * P
                    hstart = hf * 1024
                    qstart = max(k0, hstart)
                    if kb == 0:
                        avs[(h, hf)] = av_ps.tile(
                            [DH + 1, 2, 512], f32,
                            tag="av", name=f"av_{h}_{hf}")
                    av = avs[(h, hf)]
                    qpos = qstart
                    while qpos < hstart + 1024:
                        qc = qpos // 512
                        qnext = min(hstart + 1024, (qc + 1) * 512)
                        done = kb == 4 * qc + 3
                        qr = qc - 2 * hf
                        nc.tensor.matmul(
                            av[:, qr, qpos - qc * 512:qnext - qc * 512],
                            V[:, kb, h, :],
                            strip_sb[:, qpos - hstart:qnext - hstart],
                            start=(kb == 0), stop=done)
                        # h3/hf1 normalizes per 512-chunk so ZN q 1024:1536
                        # is final early (enables out-proj qt8..11 before the
                        # very end); everything else normalizes per q-half
                        # with one fused [64,1024] op chain.
                        if done and h == NH - 1 and hf == 1:
                            drow = nrm.tile([1, 512], f32, tag="dr")
                            nc.vector.tensor_copy(drow, av[DH:DH + 1, qr, :])
                            rr = nrm.tile([1, 512], f32, tag="rr")
                            nc.vector.reciprocal_approx_fast(out=rr, in_=drow)
                            rb = rrb.tile([64, 512], f32, tag="rb")
                            nc.gpsimd.partition_broadcast(rb, rr)
                            nc.vector.tensor_tensor(
                                ZN[pb:pb + 64, t, qc * 512:(qc + 1) * 512],
                                av[0:DH, qr, :], rb, mult)
                        elif done and qc % 2 == 1:
                            drow = nrm.tile([1, 1024], f32, tag="dr2")
                            nc.vector.tensor_copy(
                                drow, av[DH:DH + 1, :, :].rearrange(
                                    "p a b -> p (a b)"))
                            rr = nrm.tile([1, 1024], f32, tag="rr2")
                            nc.vector.reciprocal_approx_fast(out=rr, in_=drow)
                            rb = rrb.tile([64, 1024], f32, tag="rb2")
                            nc.gpsimd.partition_broadcast(rb, rr)
                            nc.vector.tensor_tensor(
                                ZN[pb:pb + 64, t, hstart:hstart + 1024],
                                av[0:DH, :, :].rearrange("p a b -> p (a b)"),
                                rb, mult)
                        qpos = qnext

                from collections import deque

                def emit_op(sc_ps, osb, qt, act_evac=False):
                    # output projection for one 128-row q block, borrowing a
                    # score-strip PSUM tile (keeps total PSUM at 8 banks)
                    ps = sc_ps.tile([P, 1024], f32, tag="sps",
                                    name=f"op_{qt}")
                    for dc in range(2):
                        for t in range(2):
                            nc.tensor.matmul(
                                ps[:, dc * 512:(dc + 1) * 512],
                                ZN[:, t, qt * P:(qt + 1) * P],
                                WO[:, t, dc * 512:(dc + 1) * 512],
                                start=(t == 0), stop=(t == 1))
                    ob = osb.tile([P, 1024], f16, tag="ob",
                                  name=f"ob_{qt}")
                    if act_evac:
                        # post-attention: ScalarE is idle, split the evac
                        nc.vector.tensor_copy(ob[:, 0:512], ps[:, 0:512])
                        nc.scalar.copy(ob[:, 512:1024], ps[:, 512:1024])
                    else:
                        nc.vector.tensor_copy(ob, ps)
                    oeng = (nc.sync, nc.scalar, nc.gpsimd)[qt % 3]
                    oeng.dma_start(out_d[qt * P:(qt + 1) * P, :], ob)

                def run_strips(sc_ps, av_ps, osb, work):
                    pending = deque()
                    for item in work:
                        if item[0] == "op":
                            emit_op(sc_ps, osb, item[1])
                            continue
                        sid = item[1]
                        sb_tile = emit_scores(sc_ps, *sid)
                        pending.append((sid, sb_tile))
                        if len(pending) > 2:
                            psid, psb = pending.popleft()
                            emit_av(av_ps, *psid, psb)
                    while pending:
                        psid, psb = pending.popleft()
                        emit_av(av_ps, *psid, psb)

                # hf-major strip order: all q-half-0 strips first so
                # ZN[:, :, 0:1024] completes early; out-proj q-blocks are
                # interleaved as single items every ~4 strips once their ZN
                # rows are final (the sps-slot ring then absorbs the evac
                # latency without stalling the strip pipeline).
                work = []
                for h in range(NH):
                    for kb in range(QC + 4):
                        work.append(("sc", (h, kb, 0)))
                for h in range(NH):
                    for kb in range(NT):
                        work.append(("sc", (h, kb, 1)))
                        if h <= 1 and kb % 4 == 3:
                            # qt0..7 (q first half) spread over h0/h1 groups
                            work.append(("op", 4 * h + kb // 4))
                        # h3: the qc2 norm is emitted when the kb11 strip is
                        # popped (at the kb13 push); qt 8..11 only need
                        # ZN q 1024:1536
                        if h == NH - 1 and kb >= 13:
                            work.append(("op", kb - 5))
                work.append(("op", 11))

                with tc.tile_pool(name="sc_psA", bufs=2, space="PSUM") as scA, \
                        tc.tile_pool(name="av_psA", bufs=2, space="PSUM") as avA, \
                        tc.tile_pool(name="osb", bufs=3) as osb:
                    run_strips(scA, avA, osb, work)
                    if debug:
                        for nm, tl in (("qt", QT), ("kt", KT),
                                       ("v", V), ("zn", ZN)):
                            nc.gpsimd.dma_start(
                                dbg[nm], tl.rearrange("p ... -> p (...)"))
                    # remaining out-projection (q 1536:2048)
                    for qt in range(12, NT):
                        emit_op(scA, osb, qt, act_evac=True)

    nc.compile()
    return nc


def _get_nc(debug=False):
    key = ("nc", debug)
    if key not in _CACHE:
        _CACHE[key] = _build_nc(debug)
    return _CACHE[key]


def _host_inputs(x, W_Q, W_K, W_V, W_O, b_Q, b_K, b_V):
    """Build the 8 per-core input maps (all fp16)."""
    x = np.asarray(x, dtype=np.float32)
    scale = 1.0 / np.sqrt(np.float32(DH))
    ones = np.ones((1, S), dtype=np.float16)
    vones = np.ones((P, NT * NH), dtype=np.float16)
    trim = np.where(np.arange(P)[:, None] <= np.arange(P)[None, :],
                    np.float32(0.0), np.float32(MASK_VAL)).astype(np.float16)
    iden = np.eye(P, dtype=np.float16)

    xts = [np.ascontiguousarray(x[b].T).astype(np.float16) for b in range(B)]

    in_maps = []
    for c in range(NCORES):
        b, hg = divmod(c, NCORES // B)
        h0 = NH * hg
        def chunked(a):   # [D, M] -> [128, KCH*M] with rows p, cols (ch, m)
            return np.ascontiguousarray(
                a.reshape(KCH, P, -1).transpose(1, 0, 2).reshape(P, -1)
            ).astype(np.float16)
        wq = chunked((np.asarray(W_Q[h0:h0 + NH], np.float32) * scale)
                     .reshape(NH * DH, D).T)
        wk = chunked(np.asarray(W_K[h0:h0 + NH], np.float32)
                     .reshape(NH * DH, D).T)
        wv = chunked(np.asarray(W_V[h0:h0 + NH], np.float32)
                     .reshape(NH * DH, D).T)
        wo_flat = np.asarray(W_O[h0:h0 + NH], np.float32) \
            .transpose(0, 2, 1).reshape(NH * DH, D)
        wo = np.ascontiguousarray(
            wo_flat.reshape(2, P, D).transpose(1, 0, 2).reshape(P, 2 * D)
        ).astype(np.float16)
        # per-partition bias columns: col t = heads (2t, 2t+1) x 64 dh
        bqc = np.ascontiguousarray(
            (np.asarray(b_Q[h0:h0 + NH], np.float32) * scale)
            .reshape(2, P).T).astype(np.float32)
        bkc = np.ascontiguousarray(
            np.asarray(b_K[h0:h0 + NH], np.float32)
            .reshape(2, P).T).astype(np.float32)
        bv = np.asarray(b_V[h0:h0 + NH], np.float32) \
            .reshape(1, NH * DH).astype(np.float16)
        in_maps.append({
            "xt": xts[b], "wq": wq, "wk": wk, "wv": wv, "wo": wo,
            "bqc": bqc, "bkc": bkc,
            "bv": np.ascontiguousarray(bv), "ones": ones, "vones": vones,
            "trim": trim, "iden": iden,
        })
    return in_maps


def run_spmd(in_maps, debug=False, **kwargs):
    from concourse import bass_utils
    nc = _get_nc(debug)
    return bass_utils.run_bass_kernel_spmd(
        nc, in_maps, core_ids=list(range(NCORES)), **kwargs)


def kernel(x, W_Q, W_K, W_V, W_O, b_Q, b_K, b_V, b_O):
    in_maps = _host_inputs(x, W_Q, W_K, W_V, W_O, b_Q, b_K, b_V)
    res = run_spmd(in_maps)
    parts = [res.results[c]["out"].astype(np.float32) for c in range(NCORES)]
    gpb = NCORES // B
    out = np.stack(
        [sum(parts[b * gpb + g] for g in range(gpb)) for b in range(B)], axis=0)
    out += np.asarray(b_O, np.float32)[None, None, :]
    return out.astype(np.float32)
